# revision 1
# baseline (speedup 1.0000x reference)
"""MultiHeadAttention (single-query cross-attention) Bass kernel for 8x TRN2.

Problem: B=32, S=4096, E=1024, H=16, D=64 (qk head dim), NV=64 (v head dim).
  q = seq1 @ Wq + bq                         [B,1,H*D]
  k = seq2 @ Wk + bk                         [B,S,H*D]
  v = seq2 @ Wv + bv                         [B,S,E]
  score = (q . k)/sqrt(D) per head, masked; attn = softmax(score)
  out = attn @ v                             [B,1,E]

Key algebraic rewrite (the query length is 1, so full K/V projections are
rank-wasteful):
  score[b,h,s] = sum_e seq2[b,s,e] * qk[b,h,e],  qk[b,h,:] = Wk[:,hD:hD+D] @ q[b,h,:]
  out[b,h,:]   = (attn[b,h,:] @ seq2[b]) @ Wv[:, hNV:hNV+NV] / Z
This drops the 2*B*S*E*E k/v-projection FLOPs (~550 GF) to ~35 GF total and
makes the kernel HBM-streaming-bound on seq2 (ridge regime).

bk is dropped: it shifts every score in a softmax row by the same constant
(q . bk_h), which cancels exactly in softmax (and bk is zeros in this problem).
Softmax is computed without max subtraction: scores are ~N(0,1) (|s| < ~6),
exp is safe in fp32, and softmax is shift-invariant so the result matches the
reference. Masking: exp(-1e9) underflows to exactly 0 in fp32, so multiplying
exp(score) by the {0,1} mask is exact; the mask multiply is fused into the
PSUM->SBUF evacuation of the transposed softmax weights, and the masked
normalizer Z comes from an extra N=1 matmul against a ones vector.

Sharding: data-parallel over batch, 4 batches per core (spec hint).
"""

import os
import sys
import time

import numpy as np

sys.path.insert(0, "/opt/trn_rl_repo")

import concourse.bacc as bacc
import concourse.mybir as mybir
import concourse.tile as tile
from concourse.bass_utils import run_bass_kernel_spmd

N_CORES = 8
B, S, E = 32, 4096, 1024
H, D = 16, 64
B_LOC = B // N_CORES           # 4 batches per core
CH = 128                       # seq rows per chunk (= SBUF partitions)
GRP = 4                        # chunks per score-group (score matmul N=512)
N_CHUNK = S // CH              # 32 chunks per batch
N_GRP = N_CHUNK // GRP         # 8 groups per batch

F32 = mybir.dt.float32
AF = mybir.ActivationFunctionType

# Matmul operand dtype: float32 (exact, 4 cyc/row) or float32r
# (single-pass, 1 cyc/row at N>=256). Toggled per call site below.
MM_DT = F32


def _mm(ap):
    """View an fp32 AP with the matmul dtype (same bytes)."""
    if MM_DT is F32:
        return ap
    return ap.bitcast(MM_DT)


def build_nc():
    nc = bacc.Bacc("TRN2", target_bir_lowering=False, debug=False, num_devices=1)

    seq2 = nc.dram_tensor("seq2", [B_LOC * S, E], F32, kind="ExternalInput").ap()
    seq1 = nc.dram_tensor("seq1", [B_LOC, E], F32, kind="ExternalInput").ap()
    maskt = nc.dram_tensor("maskt", [B_LOC * CH, N_CHUNK], F32, kind="ExternalInput").ap()
    wq = nc.dram_tensor("wq", [E, E], F32, kind="ExternalInput").ap()
    wk = nc.dram_tensor("wk", [E, E], F32, kind="ExternalInput").ap()
    wv = nc.dram_tensor("wv", [E, E], F32, kind="ExternalInput").ap()
    bq4 = nc.dram_tensor("bq4", [B_LOC, E], F32, kind="ExternalInput").ap()
    bv4 = nc.dram_tensor("bv4", [B_LOC, E], F32, kind="ExternalInput").ap()
    ident = nc.dram_tensor("ident", [128, 128], F32, kind="ExternalInput").ap()
    out = nc.dram_tensor("out", [B_LOC, E], F32, kind="ExternalOutput").ap()

    # Default to a linearized schedule: the parallel Tile schedule currently
    # trips a hardware fault (PSUM bank collision class); linearized runs
    # correctly. Set KLIN=0 to experiment with the parallel schedule.
    lin = os.environ.get("KLIN", "1") == "1"
    with tile.TileContext(nc, linearize=lin) as tc:
        _body(tc, seq2, seq1, maskt, wq, wk, wv, bq4, bv4, ident, out)
    nc.compile()
    return nc


def _body(tc, seq2, seq1, maskt, wq, wk, wv, bq4, bv4, ident, out):
    nc = tc.nc
    NE = E // 128  # 8 column/row blocks of the embedding dim

    from contextlib import ExitStack

    with ExitStack() as stk:
        # ---- SBUF pools ------------------------------------------------
        consts = stk.enter_context(tc.tile_pool(name="consts", bufs=1))
        bigw = stk.enter_context(tc.tile_pool(name="bigw", bufs=1))     # wq, later wv
        wkp = stk.enter_context(tc.tile_pool(name="wkp", bufs=1))       # wk natural
        wktp = stk.enter_context(tc.tile_pool(name="wktp", bufs=1))     # wk transposed
        small = stk.enter_context(tc.tile_pool(name="small", bufs=1))
        chp = stk.enter_context(tc.tile_pool(name="chp", bufs=8))       # seq2 chunks
        ctp = stk.enter_context(tc.tile_pool(name="ctp", bufs=2))       # chunkT (8 tags)
        wp = stk.enter_context(tc.tile_pool(name="wp", bufs=2))         # exp(scores)
        wtp = stk.enter_context(tc.tile_pool(name="wtp", bufs=2))       # masked wT
        outp = stk.enter_context(tc.tile_pool(name="outp", bufs=1))

        # ---- constants -------------------------------------------------
        ident_sb = consts.tile([128, 128], F32, tag="ident", name="ident")
        nc.sync.dma_start(ident_sb[:], ident[:])
        ones_sb = consts.tile([128, 1], F32, tag="ones", name="ones")
        nc.vector.memset(ones_sb[:], 1.0)
        mask_sb = []
        for b in range(B_LOC):
            m = consts.tile([CH, N_CHUNK], F32, tag=f"mask{b}", name=f"mask{b}")
            nc.sync.dma_start(m[:], maskt[b * CH:(b + 1) * CH, :])
            mask_sb.append(m)
        seq1_sb = consts.tile([B_LOC, E], F32, tag="seq1", name="seq1")
        nc.sync.dma_start(seq1_sb[:], seq1[:])
        bq4_sb = consts.tile([B_LOC, E], F32, tag="bq4", name="bq4")
        nc.sync.dma_start(bq4_sb[:], bq4[:])
        bv4_sb = consts.tile([B_LOC, E], F32, tag="bv4", name="bv4")
        nc.sync.dma_start(bv4_sb[:], bv4[:])

        wq_sb = []
        for j in range(NE):
            t = bigw.tile([128, E], F32, tag=f"bw{j}", name=f"bw{j}")
            nc.sync.dma_start(t[:], wq[j * 128:(j + 1) * 128, :])
            wq_sb.append(t)
        wk_sb = []
        for j in range(NE):
            t = wkp.tile([128, E], F32, tag=f"wk{j}", name=f"wk{j}")
            nc.sync.dma_start(t[:], wk[j * 128:(j + 1) * 128, :])
            wk_sb.append(t)

        # ---- PSUM pools (8 banks total budget) -------------------------
        tpp = stk.enter_context(tc.tile_pool(name="tpp", bufs=int(os.environ.get("KTPB", 3)), space="PSUM"))
        scp = stk.enter_context(tc.tile_pool(name="scp", bufs=1, space="PSUM"))   # 1 bank
        inner = stk.enter_context(ExitStack())
        ctxp = inner.enter_context(tc.tile_pool(name="ctxp", bufs=1, space="PSUM"))  # 3 banks
        wtpp = inner.enter_context(tc.tile_pool(name="wtpp", bufs=1, space="PSUM"))  # 1 bank

        # ================= prologue: q and qk =========================
        kprol = int(os.environ.get("KPROL", 3))
        qk_sb = []
        if kprol < 3:
            for ei in range(NE):
                t = small.tile([128, 64], F32, tag=f"qk{ei}", name=f"qk{ei}")
                nc.vector.memset(t[:], 0.001)
                qk_sb.append(t)
        # q = seq1 @ Wq + bq   -> [B_LOC, E]
        s1t = []
        for j in range(NE if kprol >= 1 else 0):
            ps = tpp.tile([128, 512], F32, tag="tp", name="tp")
            nc.tensor.transpose(ps[:, 0:B_LOC], seq1_sb[:, j * 128:(j + 1) * 128],
                                ident_sb[0:B_LOC, 0:B_LOC])
            t = small.tile([128, B_LOC], F32, tag=f"s1t{j}", name=f"s1t{j}")
            nc.vector.tensor_copy(t[:], ps[:, 0:B_LOC])
            s1t.append(t)
        q_ps = None
        if kprol >= 1:
            q_ps = ctxp.tile([B_LOC, 1536], F32, tag="ctx", name="ctx")
        for j in range(NE if kprol >= 1 else 0):
            for h in range(2):
                nc.tensor.matmul(q_ps[:, h * 512:(h + 1) * 512], _mm(s1t[j][:]),
                                 _mm(wq_sb[j][:, h * 512:(h + 1) * 512]),
                                 start=(j == 0), stop=(j == NE - 1),
                                 skip_group_check=True)
        q_sb = small.tile([B_LOC, E], F32, tag="q", name="q")
        if kprol >= 1:
            nc.vector.tensor_add(q_sb[:], q_ps[:, 0:E], bq4_sb[:])
        else:
            nc.vector.memset(q_sb[:], 0.001)
        if os.environ.get("KPART") == "q":
            nc.sync.dma_start(out[:], q_sb[:])
            return
        # qT blocks [128(hd), B_LOC]
        qt = []
        for j in range(NE if kprol >= 1 else 0):
            ps = tpp.tile([128, 512], F32, tag="tp", name="tp")
            nc.tensor.transpose(ps[:, 0:B_LOC], q_sb[:, j * 128:(j + 1) * 128],
                                ident_sb[0:B_LOC, 0:B_LOC])
            t = small.tile([128, B_LOC], F32, tag=f"qt{j}", name=f"qt{j}")
            nc.vector.tensor_copy(t[:], ps[:, 0:B_LOC])
            qt.append(t)

        # WkT: wkt[j][hd=128, e=1024] = Wk[:, 128j:128j+128].T
        wkt = []
        for j in range(NE if kprol >= 2 else 0):
            wkt.append(wktp.tile([128, E], F32, tag=f"wkt{j}", name=f"wkt{j}"))
        for ei in range(NE if kprol >= 2 else 0):
            for hj in range(NE):
                ps = tpp.tile([128, 512], F32, tag="tp", name="tp")
                nc.tensor.transpose(ps[:, 0:128],
                                    wk_sb[ei][:, hj * 128:(hj + 1) * 128],
                                    ident_sb[:])
                if (ei + hj) % 2 == 0 or os.environ.get("KEVAC") == "dve":
                    nc.vector.tensor_copy(wkt[hj][:, ei * 128:(ei + 1) * 128], ps[:, 0:128])
                else:
                    nc.scalar.copy(wkt[hj][:, ei * 128:(ei + 1) * 128], ps[:, 0:128])

        if os.environ.get("KPART") == "wkt":
            nc.sync.dma_start(out[:], wkt[0][0:B_LOC, :])
            return

        # qk_all[e, 16b+h] = sum_d Wk[e, h*64+d] * q[b, h*64+d]
        for ei in range(NE if kprol >= 3 else 0):
            ps = wtpp.tile([128, 64], F32, tag="wt", name="wt")
            psr = ps.rearrange("p (b h) -> p b h", h=H)
            for h in range(H):
                j, r = h // 2, (h % 2) * 64
                # each head writes a disjoint column set: own group
                # (start clears has_written bank-wide; data is untouched)
                nc.tensor.matmul(psr[:, :, h:h + 1],
                                 _mm(wkt[j][r:r + 64, ei * 128:(ei + 1) * 128]),
                                 _mm(qt[j][r:r + 64, :]),
                                 start=True, stop=True,
                                 skip_group_check=True)
            t = small.tile([128, 64], F32, tag=f"qk{ei}", name=f"qk{ei}")
            nc.vector.tensor_copy(t[:], ps[:])
            qk_sb.append(t)

        if os.environ.get("KPART") == "1":
            nc.sync.dma_start(out[:, 0:64], qk_sb[0][0:B_LOC, :])
            return

        # ================= main loop ==================================
        ctxn = [outp.tile([H, E], F32, tag=f"ctxn{b}", name=f"ctxn{b}")
                for b in range(B_LOC)]
        n_b = int(os.environ.get("KNB", B_LOC))
        n_g = int(os.environ.get("KNG", N_GRP))
        for b in range(n_b):
            ctx_ps = ctxp.tile([H, 1536], F32, tag="ctx", name="ctx")  # 0:1024 ctx, 1024 Z
            first = {0: True, 512: True, 1024: True}
            for g in range(n_g):
                ct = [ctp.tile([128, 512], F32, tag=f"ct{j}", name=f"ct{j}") for j in range(NE)]
                chunks = []
                for i in range(GRP):
                    c = g * GRP + i
                    r0 = b * S + g * 512 + i * CH
                    ch = chp.tile([CH, E], F32, tag="ch", name="ch")
                    nc.sync.dma_start(ch[:], seq2[r0:r0 + CH, :])
                    chunks.append(ch)
                    for half in range(2):
                        ps = tpp.tile([128, 512], F32, tag="tp", name="tp")
                        for j4 in range(4):
                            j = half * 4 + j4
                            nc.tensor.transpose(ps[:, j4 * 128:(j4 + 1) * 128],
                                                ch[:, j * 128:(j + 1) * 128],
                                                ident_sb[:])
                        for j4 in range(4):
                            j = half * 4 + j4
                            dst = ct[j][:, i * 128:(i + 1) * 128]
                            src = ps[:, j4 * 128:(j4 + 1) * 128]
                            if j % 2 == 0 or os.environ.get("KEVAC") == "dve":
                                nc.vector.tensor_copy(dst, src)
                            else:
                                nc.scalar.copy(dst, src)
                kstage = int(os.environ.get("KSTAGE", 9))
                if kstage < 1:
                    continue
                # scores [16, 512] over this group
                sc_ps = scp.tile([H, 512], F32, tag="sc", name="sc")
                for j in range(NE):
                    nc.tensor.matmul(sc_ps[:],
                                     _mm(qk_sb[j][:, b * H:(b + 1) * H]),
                                     _mm(ct[j][:]),
                                     start=(j == 0), stop=(j == NE - 1),
                                     skip_group_check=True)
                w_sb = wp.tile([H, 512], F32, tag="w", name="w")
                nc.scalar.activation(w_sb[:], sc_ps[:], AF.Exp, scale=1.0 / (D ** 0.5))
                if kstage < 2:
                    continue
                # wT per chunk, mask fused into evacuation
                wt_ps = wtpp.tile([128, 64], F32, tag="wt", name="wt")
                wt_sb = wtp.tile([128, 64], F32, tag="wts", name="wts")
                for i in range(4):
                    nc.tensor.transpose(wt_ps[:, i * 16:(i + 1) * 16],
                                        w_sb[:, i * 128:(i + 1) * 128],
                                        ident_sb[0:H, 0:H])
                for i in range(4):
                    c = g * GRP + i
                    nc.vector.tensor_scalar_mul(wt_sb[:, i * 16:(i + 1) * 16],
                                                wt_ps[:, i * 16:(i + 1) * 16],
                                                mask_sb[b][:, c:c + 1])
                if kstage < 3:
                    continue
                # ctx += wT.T @ chunk ; Z += wT.T @ ones
                for i in range(4):
                    lhs = _mm(wt_sb[:, i * 16:(i + 1) * 16])
                    last = (g == n_g - 1 and i == 3)
                    for h in range(2):
                        nc.tensor.matmul(ctx_ps[:, h * 512:(h + 1) * 512], lhs,
                                         _mm(chunks[i][:, h * 512:(h + 1) * 512]),
                                         start=first[h * 512], stop=last,
                                         skip_group_check=True)
                        first[h * 512] = False
                    nc.tensor.matmul(ctx_ps[:, 1024:1025], lhs, _mm(ones_sb[:]),
                                     start=first[1024], stop=last,
                                     skip_group_check=True)
                    first[1024] = False
            if os.environ.get("KPART") == "2":
                if int(os.environ.get("KSTAGE", 9)) >= 3:
                    ct_sb = outp.tile([H, E], F32, tag="ct_dbg", name="ct_dbg")
                    nc.vector.tensor_copy(ct_sb[:], ctx_ps[:, 0:E])
                    nc.sync.dma_start(out[:], ct_sb[0:B_LOC, :])
                else:
                    nc.sync.dma_start(out[:, 0:128], ident_sb[0:B_LOC, :])
                return
            # normalize: ctxn[b] = ctx / Z
            zr = small.tile([H, 1], F32, tag="zr", name="zr")
            nc.vector.reciprocal(zr[:], ctx_ps[:, 1024:1025])
            nc.vector.tensor_scalar_mul(ctxn[b][:], ctx_ps[:, 0:E], zr[:])
            if os.environ.get("KPART") == "3":
                nc.sync.dma_start(out[:], ctxn[0][0:B_LOC, :])
                return

        # ================= finale: out = ctxn @ Wv (head-block diag) ===
        inner.close()  # free ctxp + wtpp banks for the output pool
        opp = stk.enter_context(tc.tile_pool(name="opp", bufs=2, space="PSUM"))
        wv_sb = []
        for j in range(NE):
            t = bigw.tile([128, E], F32, tag=f"bw{j}", name=f"bw{j}")
            nc.sync.dma_start(t[:], wv[j * 128:(j + 1) * 128, :])
            wv_sb.append(t)
        cxt = []
        for j in range(NE):
            ps = tpp.tile([128, 512], F32, tag="tp", name="tp")
            for b in range(B_LOC):
                nc.tensor.transpose(ps[:, b * H:(b + 1) * H],
                                    ctxn[b][:, j * 128:(j + 1) * 128],
                                    ident_sb[0:H, 0:H])
            t = small.tile([128, B_LOC * H], F32, tag=f"cxt{j}", name=f"cxt{j}")
            nc.vector.tensor_copy(t[:], ps[:, 0:B_LOC * H])
            cxt.append(t)
        out_sb = outp.tile([B_LOC, E], F32, tag="osb", name="osb")
        for h in range(H):
            # one bank-sized tile per head: slot reuse serializes groups,
            # so no two accumulation groups ever share a live bank
            op_t = opp.tile([B_LOC, 64], F32, tag="op", name="op")
            for j in range(NE):
                lhs = cxt[j].rearrange("p (b h) -> p h b", h=H)
                nc.tensor.matmul(op_t[:], _mm(lhs[:, h:h + 1, :]),
                                 _mm(wv_sb[j][:, h * 64:(h + 1) * 64]),
                                 start=(j == 0), stop=(j == NE - 1),
                                 skip_group_check=True)
            nc.vector.tensor_add(out_sb[:, h * 64:(h + 1) * 64], op_t[:],
                                 bv4_sb[:, h * 64:(h + 1) * 64])
        nc.sync.dma_start(out[:], out_sb[:])


_NC_CACHE = None

# test-harness knobs (the grading harness never touches these)
TRACE = False
TRACE_DIR = None
LAST_RESULTS = None


def _get_nc():
    global _NC_CACHE
    if _NC_CACHE is None:
        _NC_CACHE = build_nc()
    return _NC_CACHE


def kernel(**inputs):
    seq1 = np.asarray(inputs["seq1"], dtype=np.float32)   # [B,1,E]
    seq2 = np.asarray(inputs["seq2"], dtype=np.float32)   # [B,S,E]
    mask = np.asarray(inputs["mask"])                     # [B,1,1,S] int32
    Wq = np.asarray(inputs["Wq"], dtype=np.float32)
    bq = np.asarray(inputs["bq"], dtype=np.float32)
    Wk = np.asarray(inputs["Wk"], dtype=np.float32)
    # bk dropped: uniform per-row score shift, cancels exactly in softmax.
    Wv = np.asarray(inputs["Wv"], dtype=np.float32)
    bv = np.asarray(inputs["bv"], dtype=np.float32)

    nc = _get_nc()

    ident = np.eye(128, dtype=np.float32)
    bq4 = np.tile(bq[None, :], (B_LOC, 1)).astype(np.float32)
    bv4 = np.tile(bv[None, :], (B_LOC, 1)).astype(np.float32)
    # mask -> fp32 {0,1}, laid out [B_LOC*128, 32]: [p, c] = mask[b, c*128+p]
    mf = (mask.reshape(B, S) != 0).astype(np.float32)

    in_maps = []
    for core in range(N_CORES):
        b0 = core * B_LOC
        mt = mf[b0:b0 + B_LOC].reshape(B_LOC, N_CHUNK, CH).transpose(0, 2, 1)
        in_maps.append({
            "seq2": np.ascontiguousarray(seq2[b0:b0 + B_LOC].reshape(B_LOC * S, E)),
            "seq1": np.ascontiguousarray(seq1[b0:b0 + B_LOC, 0, :]),
            "maskt": np.ascontiguousarray(mt.reshape(B_LOC * CH, N_CHUNK)),
            "wq": Wq, "wk": Wk, "wv": Wv,
            "bq4": bq4, "bv4": bv4, "ident": ident,
        })

    global LAST_RESULTS
    kwargs = {}
    if TRACE:
        kwargs = {"trace": True, "tmpdir": TRACE_DIR}
    # Retry: a previously-faulted NeuronCore can be left wedged
    # (NRT_EXEC_UNIT_UNRECOVERABLE) and recovers after reset/re-init.
    last_exc = None
    for attempt in range(4):
        try:
            res = run_bass_kernel_spmd(nc, in_maps, list(range(N_CORES)), **kwargs)
            break
        except Exception as e:  # noqa: BLE001
            last_exc = e
            time.sleep(10 * (attempt + 1))
    else:
        raise last_exc
    LAST_RESULTS = res
    out = np.concatenate([res.results[c]["out"] for c in range(N_CORES)], axis=0)
    return out.reshape(B, 1, E)


if __name__ == "__main__":
    t0 = time.time()
    nc = build_nc()
    print(f"build+compile(py): {time.time() - t0:.1f}s")



# revision 7
# speedup vs baseline: 7.0993x; 7.0993x over previous
"""MultiHeadAttention (single-query cross-attention) Bass kernel for 8x TRN2.

Problem: B=32, S=4096, E=1024, H=16, D=64 (qk head dim), NV=64 (v head dim).
  q = seq1 @ Wq + bq                         [B,1,H*D]
  k = seq2 @ Wk + bk                         [B,S,H*D]
  v = seq2 @ Wv + bv                         [B,S,E]
  score = (q . k)/sqrt(D) per head, masked; attn = softmax(score)
  out = attn @ v                             [B,1,E]

Algebraic rewrite (query length is 1, so full K/V projections are
rank-wasteful):
  score[b,h,s] = sum_e seq2[b,s,e] * qk[b,h,e],  qk[b,h,:] = Wk[:,hD:hD+D] @ q[b,h,:]
  out[b,h,:]   = (attn[b,h,:] @ seq2[b]) @ Wv[:, hNV:hNV+NV] / Z
This drops the 2*B*S*E*E k/v-projection FLOPs (~550 GF) to ~35 GF total and
makes the kernel HBM-streaming-bound on seq2 (ridge regime).

bk is dropped: it shifts every score in a softmax row by the same constant,
which cancels exactly in softmax. Softmax runs without max subtraction:
scores are ~N(0,1), exp is safe, softmax is shift-invariant. Masking:
exp(score)*mask with mask in {0,1} is exact; the masked normalizer Z comes
from an extra N=1 matmul against a ones vector.

v2 design:
- All matmul operands bf16 (host-side cast of seq2/W*; fp32 PSUM accumulate).
  1 cyc/row on PE instead of fp32's 4.
- Host supplies Wk^T, seq1^T and (optionally, KF e-blocks of) seq2^T --
  pure layout transforms of inputs, so prologue/main-loop PE transposes and
  their PSUM evacuations disappear in favor of idle DMA bandwidth.
- qk and the output projection computed as wide N=512 accumulating matmuls
  (stationary operand <= 64 cols) instead of per-head LDWEIGHTS-bound ones.
- Combined (head,batch) index order is h*4+b everywhere.

Sharding: data-parallel over batch, 4 batches per core (spec hint).
"""

import os
import sys
import time

import numpy as np

sys.path.insert(0, "/opt/trn_rl_repo")

import concourse.bacc as bacc
import concourse.mybir as mybir
import concourse.tile as tile
from concourse.bass_utils import run_bass_kernel_spmd

N_CORES = 8
B, S, E = 32, 4096, 1024
H, D = 16, 64
B_LOC = B // N_CORES           # 4 batches per core
CH = 128                       # seq rows per chunk (= SBUF partitions)
GRP = 4                        # chunks per score-group (score matmul N=512)
N_CHUNK = S // CH              # 32 chunks per batch
N_GRP = N_CHUNK // GRP         # 8 groups per batch
NE = E // 128                  # 8 e-blocks

F32 = mybir.dt.float32
BF16 = mybir.dt.bfloat16
AF = mybir.ActivationFunctionType

# Number of e-blocks whose transposed seq2 comes pre-staged from the host
# (the rest are transposed on the PE). Balances DMA vs tensor engine.
KF = int(os.environ.get("KF", 4))
J0 = NE - KF                   # host-fed block indices are J0..NE-1


def build_nc():
    nc = bacc.Bacc("TRN2", target_bir_lowering=False, debug=False, num_devices=1)

    seq2 = nc.dram_tensor("seq2", [B_LOC * S, E], BF16, kind="ExternalInput").ap()
    if KF > 0:
        seq2t = nc.dram_tensor("seq2t", [B_LOC * KF * 128, S], BF16,
                               kind="ExternalInput").ap()
    else:
        seq2t = None
    seq1t = nc.dram_tensor("seq1t", [128, NE * B_LOC], BF16, kind="ExternalInput").ap()
    maskt = nc.dram_tensor("maskt", [B_LOC * CH, N_CHUNK], F32, kind="ExternalInput").ap()
    wq = nc.dram_tensor("wq", [E, E], BF16, kind="ExternalInput").ap()
    wkt = nc.dram_tensor("wkt", [E, E], BF16, kind="ExternalInput").ap()
    wv = nc.dram_tensor("wv", [E, E], BF16, kind="ExternalInput").ap()
    bq4 = nc.dram_tensor("bq4", [B_LOC, E], F32, kind="ExternalInput").ap()
    bv64 = nc.dram_tensor("bv64", [64, E], F32, kind="ExternalInput").ap()
    ident = nc.dram_tensor("ident", [128, 128], BF16, kind="ExternalInput").ap()
    out = nc.dram_tensor("out", [B_LOC, E], F32, kind="ExternalOutput").ap()

    lin = os.environ.get("KLIN", "0") == "1"
    with tile.TileContext(nc, linearize=lin) as tc:
        _body(tc, seq2, seq2t, seq1t, maskt, wq, wkt, wv, bq4, bv64, ident, out)
    nc.compile()
    return nc


def _body(tc, seq2, seq2t, seq1t, maskt, wq, wkt, wv, bq4, bv64, ident, out):
    nc = tc.nc

    from contextlib import ExitStack

    with ExitStack() as stk:
        # ---- SBUF pools ------------------------------------------------
        consts = stk.enter_context(tc.tile_pool(name="consts", bufs=1))
        bigw = stk.enter_context(tc.tile_pool(name="bigw", bufs=1))    # wq, later wv
        wktp = stk.enter_context(tc.tile_pool(name="wktp", bufs=1))    # wkT
        small = stk.enter_context(tc.tile_pool(name="small", bufs=1))
        chp = stk.enter_context(tc.tile_pool(name="chp", bufs=8))      # seq2 chunks
        ctp = stk.enter_context(tc.tile_pool(name="ctp", bufs=2))      # chunkT (8 tags)
        wp = stk.enter_context(tc.tile_pool(name="wp", bufs=2))        # exp(scores)
        wtp = stk.enter_context(tc.tile_pool(name="wtp", bufs=2))      # masked wT
        outp = stk.enter_context(tc.tile_pool(name="outp", bufs=1))

        # ---- constants -------------------------------------------------
        ident_sb = consts.tile([128, 128], BF16, tag="ident", name="ident")
        nc.sync.dma_start(ident_sb[:], ident[:])
        ones_sb = consts.tile([128, 1], BF16, tag="ones", name="ones")
        nc.vector.memset(ones_sb[:], 1.0)
        mask_sb = []
        for b in range(B_LOC):
            m = consts.tile([CH, N_CHUNK], F32, tag=f"mask{b}", name=f"mask{b}")
            nc.sync.dma_start(m[:], maskt[b * CH:(b + 1) * CH, :])
            mask_sb.append(m)
        seq1t_sb = consts.tile([128, NE * B_LOC], BF16, tag="seq1t", name="seq1t")
        nc.sync.dma_start(seq1t_sb[:], seq1t[:])
        bq4_sb = consts.tile([B_LOC, E], F32, tag="bq4", name="bq4")
        nc.sync.dma_start(bq4_sb[:], bq4[:])
        bv64_sb = consts.tile([64, E], F32, tag="bv64", name="bv64")
        nc.sync.dma_start(bv64_sb[:], bv64[:])

        wq_sb = []
        for j in range(NE):
            t = bigw.tile([128, E], BF16, tag=f"bw{j}", name=f"bw{j}")
            nc.sync.dma_start(t[:], wq[j * 128:(j + 1) * 128, :])
            wq_sb.append(t)
        wkt_sb = []
        for j in range(NE):
            t = wktp.tile([128, E], BF16, tag=f"wkt{j}", name=f"wkt{j}")
            nc.sync.dma_start(t[:], wkt[j * 128:(j + 1) * 128, :])
            wkt_sb.append(t)

        # ---- PSUM pools (8 banks total) --------------------------------
        tpp = stk.enter_context(tc.tile_pool(name="tpp", bufs=3, space="PSUM"))
        scp = stk.enter_context(tc.tile_pool(name="scp", bufs=1, space="PSUM"))
        inner = stk.enter_context(ExitStack())
        ctxp = inner.enter_context(tc.tile_pool(name="ctxp", bufs=1, space="PSUM"))
        wtpp = inner.enter_context(tc.tile_pool(name="wtpp", bufs=1, space="PSUM"))

        # ================= prologue: q and qk =========================
        # q[b, hd] = sum_e seq1[b, e] Wq[e, hd] + bq
        q_ps = ctxp.tile([B_LOC, E], F32, tag="ctx", name="ctx")
        for j in range(NE):
            for h2 in range(2):
                nc.tensor.matmul(q_ps[:, h2 * 512:(h2 + 1) * 512],
                                 seq1t_sb[:, j * B_LOC:(j + 1) * B_LOC],
                                 wq_sb[j][:, h2 * 512:(h2 + 1) * 512],
                                 start=(j == 0), stop=(j == NE - 1),
                                 skip_group_check=True)
        q_sb = small.tile([B_LOC, E], BF16, tag="q", name="q")
        nc.vector.tensor_add(q_sb[:], q_ps[:, 0:E], bq4_sb[:])
        if os.environ.get("KPART") == "q":
            dbg = outp.tile([B_LOC, E], F32, tag="dbg", name="dbg")
            nc.vector.tensor_copy(dbg[:], q_sb[:])
            nc.sync.dma_start(out[:], dbg[:])
            return

        # qT blocks: qt_sb[p, j*4+b] = q[b, j*128+p]
        ps = tpp.tile([128, 1024], BF16, tag="tp", name="tp")
        for j in range(NE):
            nc.tensor.transpose(ps[:, j * B_LOC:(j + 1) * B_LOC],
                                q_sb[:, j * 128:(j + 1) * 128],
                                ident_sb[0:B_LOC, 0:B_LOC])
        qt_sb = small.tile([128, NE * B_LOC], BF16, tag="qt", name="qt")
        nc.vector.tensor_copy(qt_sb[:], ps[:, 0:NE * B_LOC])

        # block-diagonal q: qbd[j2][p, h*4+b] = qt[p, j2*4+b] iff h == 2*j2 + p//64
        qbd_sb = []
        for j2 in range(NE):
            t = small.tile([128, 64], BF16, tag=f"qbd{j2}", name=f"qbd{j2}")
            nc.vector.memset(t[:], 0.0)
            qbd_sb.append(t)
        for j2 in range(NE):
            for half in range(2):
                h = 2 * j2 + half
                nc.vector.tensor_copy(
                    qbd_sb[j2][half * 64:(half + 1) * 64, h * 4:h * 4 + 4],
                    qt_sb[half * 64:(half + 1) * 64, j2 * 4:(j2 + 1) * 4])

        # qkT[h*4+b, e] = sum_hd qbd[hd, h*4+b] * WkT[hd, e]
        qkt_ps = ctxp.tile([64, E], F32, tag="ctx", name="ctx")
        for j2 in range(NE):
            for h2 in range(2):
                nc.tensor.matmul(qkt_ps[:, h2 * 512:(h2 + 1) * 512],
                                 qbd_sb[j2][:],
                                 wkt_sb[j2][:, h2 * 512:(h2 + 1) * 512],
                                 start=(j2 == 0), stop=(j2 == NE - 1),
                                 skip_group_check=True)
        qkt_sb = small.tile([64, E], BF16, tag="qkt", name="qkt")
        nc.scalar.copy(qkt_sb[:], qkt_ps[:])

        # qk blocks: qk_sb[p, j*64 + h*4 + b] = qkT[h*4+b, j*128+p]
        ps2 = tpp.tile([128, 1024], BF16, tag="tp", name="tp")
        for j in range(NE):
            nc.tensor.transpose(ps2[:, j * 64:(j + 1) * 64],
                                qkt_sb[:, j * 128:(j + 1) * 128],
                                ident_sb[0:64, 0:64])
        qk_sb = small.tile([128, 512], BF16, tag="qk", name="qk")
        nc.vector.tensor_copy(qk_sb[:], ps2[:, 0:512])
        qkv = qk_sb.rearrange("p (j h b) -> p j h b", j=NE, h=H, b=B_LOC)

        if os.environ.get("KPART") == "1":
            dbg = outp.tile([B_LOC, E], F32, tag="dbg", name="dbg")
            nc.vector.tensor_copy(dbg[0:B_LOC, 0:512], qk_sb[0:B_LOC, :])
            nc.vector.memset(dbg[:, 512:1024], 0.0)
            nc.sync.dma_start(out[:], dbg[:])
            return

        # ================= main loop ==================================
        ctxn = [outp.tile([H, E], BF16, tag=f"ctxn{b}", name=f"ctxn{b}")
                for b in range(B_LOC)]
        n_b = int(os.environ.get("KNB", B_LOC))
        n_g = int(os.environ.get("KNG", N_GRP))
        kstage = int(os.environ.get("KSTAGE", 9))
        for b in range(n_b):
            ctx_ps = ctxp.tile([H, 1536], F32, tag="ctx", name="ctx")  # 0:1024 ctx, 1024 Z
            first = {0: True, 512: True, 1024: True}
            for g in range(n_g):
                ct = [ctp.tile([128, 512], BF16, tag=f"ct{j}", name=f"ct{j}")
                      for j in range(NE)]
                chunks = []
                for i in range(GRP):
                    r0 = b * S + g * 512 + i * CH
                    ch = chp.tile([CH, E], BF16, tag="ch", name="ch")
                    nc.sync.dma_start(ch[:], seq2[r0:r0 + CH, :])
                    chunks.append(ch)
                # host-fed transposed blocks
                for j in range(J0, NE):
                    r0 = (b * KF + (j - J0)) * 128
                    nc.sync.dma_start(ct[j][:], seq2t[r0:r0 + 128,
                                                      g * 512:(g + 1) * 512])
                # device transposes for blocks 0..J0-1 (pairs share a PSUM bank)
                for jp in range((J0 + 1) // 2):
                    pst = tpp.tile([128, 1024], BF16, tag="tp", name="tp")
                    for half in range(2):
                        j = 2 * jp + half
                        if j >= J0:
                            break
                        for i in range(GRP):
                            nc.tensor.transpose(
                                pst[:, half * 512 + i * 128:half * 512 + (i + 1) * 128],
                                chunks[i][:, j * 128:(j + 1) * 128],
                                ident_sb[:])
                    for half in range(2):
                        j = 2 * jp + half
                        if j >= J0:
                            break
                        src = pst[:, half * 512:(half + 1) * 512]
                        if j % 2 == 0:
                            nc.vector.tensor_copy(ct[j][:], src)
                        else:
                            nc.scalar.copy(ct[j][:], src)
                if kstage < 1:
                    continue
                # scores [16, 512] over this group
                sc_ps = scp.tile([H, 512], F32, tag="sc", name="sc")
                for j in range(NE):
                    nc.tensor.matmul(sc_ps[:], qkv[:, j:j + 1, :, b:b + 1],
                                     ct[j][:],
                                     start=(j == 0), stop=(j == NE - 1),
                                     skip_group_check=True)
                w_sb = wp.tile([H, 512], BF16, tag="w", name="w")
                nc.scalar.activation(w_sb[:], sc_ps[:], AF.Exp, scale=1.0 / (D ** 0.5))
                if kstage < 2:
                    continue
                # wT per chunk, mask fused into evacuation
                wt_ps = wtpp.tile([128, 64], BF16, tag="wt", name="wt")
                wt_sb = wtp.tile([128, 64], BF16, tag="wts", name="wts")
                for i in range(GRP):
                    nc.tensor.transpose(wt_ps[:, i * 16:(i + 1) * 16],
                                        w_sb[:, i * 128:(i + 1) * 128],
                                        ident_sb[0:H, 0:H])
                for i in range(GRP):
                    c = g * GRP + i
                    nc.vector.tensor_scalar_mul(wt_sb[:, i * 16:(i + 1) * 16],
                                                wt_ps[:, i * 16:(i + 1) * 16],
                                                mask_sb[b][:, c:c + 1])
                if kstage < 3:
                    continue
                # ctx += wT.T @ chunk ; Z += wT.T @ ones
                for i in range(GRP):
                    lhs = wt_sb[:, i * 16:(i + 1) * 16]
                    last = (g == n_g - 1 and i == GRP - 1)
                    for h2 in range(2):
                        nc.tensor.matmul(ctx_ps[:, h2 * 512:(h2 + 1) * 512], lhs,
                                         chunks[i][:, h2 * 512:(h2 + 1) * 512],
                                         start=first[h2 * 512], stop=last,
                                         skip_group_check=True)
                        first[h2 * 512] = False
                    nc.tensor.matmul(ctx_ps[:, 1024:1025], lhs, ones_sb[:],
                                     start=first[1024], stop=last,
                                     skip_group_check=True)
                    first[1024] = False
            if os.environ.get("KPART") == "2":
                dbg = outp.tile([B_LOC, E], F32, tag="dbg", name="dbg")
                if kstage >= 3:
                    nc.vector.tensor_copy(dbg[:], ctx_ps[0:B_LOC, 0:E])
                else:
                    nc.vector.memset(dbg[:], 0.5)
                nc.sync.dma_start(out[:], dbg[:])
                return
            # normalize: ctxn[b] = ctx / Z
            zr = small.tile([H, 1], F32, tag="zr", name="zr")
            nc.vector.reciprocal(zr[:], ctx_ps[:, 1024:1025])
            nc.vector.tensor_scalar_mul(ctxn[b][:], ctx_ps[:, 0:E], zr[:])
            if os.environ.get("KPART") == "3":
                dbg = outp.tile([B_LOC, E], F32, tag="dbg", name="dbg")
                nc.vector.tensor_copy(dbg[:], ctxn[0][0:B_LOC, :])
                nc.sync.dma_start(out[:], dbg[:])
                return

        # ================= finale: out = ctxn @ Wv (head-block diag) ===
        inner.close()  # free ctxp + wtpp banks for the output pool
        opp = stk.enter_context(tc.tile_pool(name="opp", bufs=1, space="PSUM"))
        wv_sb = []
        for j in range(NE):
            t = bigw.tile([128, E], BF16, tag=f"bw{j}", name=f"bw{j}")
            nc.sync.dma_start(t[:], wv[j * 128:(j + 1) * 128, :])
            wv_sb.append(t)
        # cxt[p, j*64 + h*4 + b] = ctxn[b][h, j*128+p]; transposes drain
        # contiguous (j,b)-blocks, the evacuation permutes (j b h)->(j h b)
        ps3 = tpp.tile([128, 1024], BF16, tag="tp", name="tp")
        for j in range(NE):
            for b in range(B_LOC):
                nc.tensor.transpose(ps3[:, j * 64 + b * 16:j * 64 + (b + 1) * 16],
                                    ctxn[b][:, j * 128:(j + 1) * 128],
                                    ident_sb[0:H, 0:H])
        cxt_sb = outp.tile([128, 512], BF16, tag="cxt", name="cxt")
        nc.vector.tensor_copy(
            cxt_sb.rearrange("p (j h b) -> p j b h", j=NE, h=H, b=B_LOC),
            ps3[:, 0:512].rearrange("p (j b h) -> p j b h", j=NE, b=B_LOC, h=H))
        # full[h*4+b, eo] = sum_e cxt[e, j*64+h*4+b] Wv[e, eo]
        full_ps = opp.tile([64, E], F32, tag="op", name="op")
        for j in range(NE):
            for h2 in range(2):
                nc.tensor.matmul(full_ps[:, h2 * 512:(h2 + 1) * 512],
                                 cxt_sb[:, j * 64:(j + 1) * 64],
                                 wv_sb[j][:, h2 * 512:(h2 + 1) * 512],
                                 start=(j == 0), stop=(j == NE - 1),
                                 skip_group_check=True)
        out_sb = outp.tile([64, E], F32, tag="osb", name="osb")
        nc.vector.tensor_add(out_sb[:], full_ps[:], bv64_sb[:])
        # diagonal head-block extraction: out[b, h*64+d] = out_sb[h*4+b, h*64+d]
        for h in range(H):
            nc.sync.dma_start(out[:, h * 64:(h + 1) * 64],
                              out_sb[h * B_LOC:(h + 1) * B_LOC,
                                     h * 64:(h + 1) * 64])


_NC_CACHE = None

# test-harness knobs (the grading harness never touches these)
TRACE = False
TRACE_DIR = None
LAST_RESULTS = None


def _get_nc():
    global _NC_CACHE
    if _NC_CACHE is None:
        _NC_CACHE = build_nc()
    return _NC_CACHE


def kernel(**inputs):
    import ml_dtypes
    bf = ml_dtypes.bfloat16

    seq1 = np.asarray(inputs["seq1"], dtype=np.float32)   # [B,1,E]
    seq2 = np.asarray(inputs["seq2"], dtype=np.float32)   # [B,S,E]
    mask = np.asarray(inputs["mask"])                     # [B,1,1,S] int32
    Wq = np.asarray(inputs["Wq"], dtype=np.float32)
    bq = np.asarray(inputs["bq"], dtype=np.float32)
    Wk = np.asarray(inputs["Wk"], dtype=np.float32)
    # bk dropped: uniform per-row score shift, cancels exactly in softmax.
    Wv = np.asarray(inputs["Wv"], dtype=np.float32)
    bv = np.asarray(inputs["bv"], dtype=np.float32)

    nc = _get_nc()

    seq2b = seq2.astype(bf)                               # [B,S,E] bf16
    wqb = Wq.astype(bf)
    wktb = np.ascontiguousarray(Wk.T).astype(bf)
    wvb = Wv.astype(bf)
    identb = np.eye(128, dtype=bf)
    bq4 = np.tile(bq[None, :], (B_LOC, 1)).astype(np.float32)
    bv64 = np.tile(bv[None, :], (64, 1)).astype(np.float32)
    # mask -> bf16 {0,1}, laid out [B_LOC*128, 32]: [p, c] = mask[b, c*128+p]
    mf = (mask.reshape(B, S) != 0)

    in_maps = []
    for core in range(N_CORES):
        b0 = core * B_LOC
        mt = mf[b0:b0 + B_LOC].reshape(B_LOC, N_CHUNK, CH).transpose(0, 2, 1)
        # seq1t[p, j*4+b] = seq1[b0+b, j*128+p]
        s1t = seq1[b0:b0 + B_LOC, 0, :].astype(bf).T.reshape(NE, 128, B_LOC)
        s1t = np.ascontiguousarray(s1t.transpose(1, 0, 2)).reshape(128, NE * B_LOC)
        m = {
            "seq2": np.ascontiguousarray(seq2b[b0:b0 + B_LOC]).reshape(B_LOC * S, E),
            "seq1t": np.ascontiguousarray(s1t),
            "maskt": mt.reshape(B_LOC * CH, N_CHUNK).astype(np.float32),
            "wq": wqb, "wkt": wktb, "wv": wvb,
            "bq4": bq4, "bv64": bv64, "ident": identb,
        }
        if KF > 0:
            # seq2t rows: b*KF*128 + (j-J0)*128 + p ; cols: s
            st = seq2b[b0:b0 + B_LOC].transpose(0, 2, 1)[:, J0 * 128:, :]
            m["seq2t"] = np.ascontiguousarray(st).reshape(B_LOC * KF * 128, S)
        in_maps.append(m)

    global LAST_RESULTS
    kwargs = {}
    if TRACE:
        kwargs = {"trace": True, "tmpdir": TRACE_DIR}
    # Retry: a previously-faulted NeuronCore can be left wedged
    # (NRT_EXEC_UNIT_UNRECOVERABLE) and recovers after reset/re-init.
    last_exc = None
    for attempt in range(int(os.environ.get("KRETRY", 4))):
        try:
            res = run_bass_kernel_spmd(nc, in_maps, list(range(N_CORES)), **kwargs)
            break
        except Exception as e:  # noqa: BLE001
            last_exc = e
            time.sleep(10 * (attempt + 1))
    else:
        raise last_exc
    LAST_RESULTS = res
    out = np.concatenate([res.results[c]["out"] for c in range(N_CORES)], axis=0)
    return out.reshape(B, 1, E)


if __name__ == "__main__":
    t0 = time.time()
    nc = build_nc()
    print(f"build+compile(py): {time.time() - t0:.1f}s")
